# revision 2
# baseline (speedup 1.0000x reference)
"""DialogueGCN windowed-attention relational GCN on 8 Trainium2 NeuronCores.

Sharding: utterance axis N=16384 split into 8 shards of 2048 rows; each core
gets its shard plus a 128-row halo on each side (zero-padded at the global
edges). Weights replicated; no collectives.

Per-core v2 design (all banded ops as dense matmuls on a 64-row-shifted
chunk grid; attention strips are produced TRANSPOSED by the R matmuls so no
PE transposes / strip copies / row-max are needed):
  qT      = (x @ W_att)^T               (wq chunks vs xts, evac to qT_pad)
  S[r]    = x_halo @ Wr_combined        (3 supports; S_a carries ones-col)
  per shifted chunk c (17 of them, serving blocks c-1 and c):
    Rc    = xts_c^T @ qT_pad[span] + bias    ([jj, i] layout, 2 blocks wide;
             bias = -SHIFT in band, -1e30 out of band -> masked softmax exp)
    ee    = exp(Rc)  (Act, bf16)        == band-masked attention numerator
    c2    = ee * sucm                   (DVE, direction mask)
    c3    = (spk_j == spk_i) * ee       (DVE stt, speaker mask)
  per block b: 6 accumulating matmuls strips^T-contract S -> psh[257]
    (col 256 = softmax denominator); tail: rinv, e2=exp(h*rinv) bf16,
    s2 = reduce_sum(e2) on DVE, pairwise ln/ob finalize on Act, DMA out.
  PE warmup matmuls on a memset tile burn the p-state ramp during the DMA
  prologue.
"""

import os
import numpy as np

N_TOT, D, W, SPK = 16384, 256, 64, 8
NCORES = 8
NC_ROWS = N_TOT // NCORES          # 2048 rows per core
HALO = 128
NH = NC_ROWS + 2 * HALO            # 2304 rows with halo
NBLK = NC_ROWS // 128              # 16 output blocks per core
NCH = NH // 128                    # 18 halo chunks (aligned grid)
NSH = NCH - 1                      # 17 chunks on the 64-shifted grid
SHIFT = float(os.environ.get("KB_SHIFT", "64.0"))
NEG_BIG = -1.0e30

_cache = {}


def _build_bass():
    import concourse.tile as tile
    from concourse import bacc, mybir

    f32 = mybir.dt.float32
    f32r = mybir.dt.float32r
    bf16 = mybir.dt.bfloat16
    AX = mybir.AxisListType.X
    OP = mybir.AluOpType
    AF = mybir.ActivationFunctionType

    nc = bacc.Bacc("TRN2", target_bir_lowering=False, debug=False,
                   num_devices=NCORES)

    xt_d = nc.dram_tensor("xt", [2, 128, NH], f32r, kind="ExternalInput").ap()
    wq_d = nc.dram_tensor("wq", [D, D], f32r, kind="ExternalInput").ap()
    wabc_d = nc.dram_tensor("wabc", [3, D, D], f32r, kind="ExternalInput").ap()
    auxf_d = nc.dram_tensor("auxf", [128, 401], f32r, kind="ExternalInput").ap()
    auxb_d = nc.dram_tensor("auxb", [128, 2560], bf16, kind="ExternalInput").ap()
    out_d = nc.dram_tensor("out", [NC_ROWS, D], f32, kind="ExternalOutput").ap()

    with tile.TileContext(nc) as tc:
        from contextlib import ExitStack
        with ExitStack() as ctx:
            const = ctx.enter_context(tc.tile_pool(name="const", bufs=1))
            persist = ctx.enter_context(tc.tile_pool(name="persist", bufs=1))
            work = ctx.enter_context(tc.tile_pool(
                name="work", bufs=int(os.environ.get("KB_WORK", "5"))))
            psum = ctx.enter_context(tc.tile_pool(name="psum", bufs=2, space="PSUM"))

            # one activation table for the whole kernel (exp/ln/copy/identity)
            nc.scalar.add_instruction(mybir.InstLoadActFuncSet(
                name=nc.get_next_instruction_name(), ins=[], outs=[],
                act_func_set_id=6))

            # ---- warmup scaffolding: Pool memset, PE burns p-state ramp ----
            zt = const.tile([128, 512], bf16)
            nc.gpsimd.memset(zt, 0.0)
            qT = persist.tile([128, 2, 2304], f32r)   # 128 pad | 2048 | 128 pad
            zf = const.tile([128, 2, 128], f32)
            nc.gpsimd.memset(zf, 0.0)
            nc.vector.tensor_copy(qT[:, :, 0:128], zf)
            nc.vector.tensor_copy(qT[:, :, 2176:2304], zf)

            NWARM = int(os.environ.get("KB_WARM", "4"))
            for i in range(NWARM):
                psw = psum.tile([128, 512], f32, tag="big", name="psw",
                                bufs=int(os.environ.get("KB_BIG", "4")))
                nc.tensor.matmul(psw, zt[:, 0:128], zt, start=True, stop=True)

            # ---- DMA order tuned for the startup critical path ----
            wq_sb = const.tile([128, 2, D], f32r)
            nc.sync.dma_start(wq_sb, wq_d.rearrange("(k p) d -> p k d", p=128))
            auxf = const.tile([128, 401], f32r)
            nc.sync.dma_start(auxf, auxf_d)
            xts = persist.tile([128, 2, NH], f32r)
            xt_v = xt_d.rearrange("k p n -> p k n")
            ngrp = NH // 256  # 9 groups of 256 cols
            nsplit = int(os.environ.get("KB_XSPLIT", "2"))
            for g in range(nsplit):
                nc.sync.dma_start(xts[:, :, g * 256:(g + 1) * 256],
                                  xt_v[:, :, g * 256:(g + 1) * 256])
            wabc_sb = const.tile([128, 3, 2, D], f32r)
            nc.sync.dma_start(
                wabc_sb, wabc_d.rearrange("w (k p) d -> p w k d", p=128))
            auxb = const.tile([128, 2560], bf16)
            nc.sync.dma_start(auxb[:, 0:1280], auxb_d[:, 0:1280])
            for g in range(nsplit, ngrp):
                nc.sync.dma_start(xts[:, :, g * 256:(g + 1) * 256],
                                  xt_v[:, :, g * 256:(g + 1) * 256])
                if g == nsplit + 1:
                    nc.sync.dma_start(auxb[:, 1280:2560], auxb_d[:, 1280:2560])

            bias_sb = auxf[:, 0:256]
            ident_sb = auxf[:, 256:384]
            spk_col = auxf[:, 384:401].bitcast(f32)
            sucm_sb = auxb[:, 0:256]
            spk_bc = auxb[:, 256:2560]   # [128, 2304], col t = shard row t-128

            S = persist.tile([128, 3, NSH, 264], bf16)
            nc.gpsimd.memset(S[:, 0, :, 256:257], 1.0)

            s2_all = persist.tile([128, NBLK], f32)
            rinv_all = persist.tile([128, NBLK], f32)

            # per-chunk strip tiles live until both consumer blocks aggregate
            nstrip = int(os.environ.get("KB_STRIP", "4"))

            evac_rot = {"i": 0}
            rot = tuple(os.environ.get("KB_EVROT", "act,dve").split(","))

            def evac(dst, src, kind):
                """PSUM evacuation with engine rotation."""
                mode = os.environ.get("KB_EV_" + kind, None)
                if mode is None:
                    i = evac_rot["i"]
                    evac_rot["i"] += 1
                    mode = rot[i % len(rot)]
                if mode == "act":
                    nc.scalar.copy(dst, src)
                elif mode == "pool":
                    nc.gpsimd.tensor_copy(dst, src)
                else:
                    nc.vector.tensor_copy(dst, src)

            # ---- qT groups (first two narrow so PE starts off 2 DMAs) ----
            QGRP = [(0, 256), (256, 256), (512, 512), (1024, 512), (1536, 512)]
            BLK2QG = [0, 0, 1, 1, 2, 2, 2, 2, 3, 3, 3, 3, 4, 4, 4, 4]

            def emit_qT(g):
                c0, w = QGRP[g]
                nsl = slice(HALO + c0, HALO + c0 + w)
                for dh in (0, 1):
                    psq = psum.tile([128, 512], f32, tag="big", name="psq",
                                    bufs=int(os.environ.get("KB_BIG", "4")))
                    for k in (0, 1):
                        nc.tensor.matmul(
                            psq[:, 0:w],
                            wq_sb[:, k, dh * 128:(dh + 1) * 128],
                            xts[:, k, nsl],
                            start=(k == 0), stop=(k == 1))
                    evac(qT[:, dh, 128 + c0:128 + c0 + w], psq[:, 0:w], "QT")

            # ---- one support chunk on the 64-shifted grid ----
            def emit_S(c):
                csl = slice(64 + c * 128, 64 + (c + 1) * 128)
                pab = psum.tile([128, 512], f32, tag="big", name="pab",
                                bufs=int(os.environ.get("KB_BIG", "4")))
                for i in (0, 1):
                    for k in (0, 1):
                        nc.tensor.matmul(
                            pab[:, i * 256:(i + 1) * 256],
                            xts[:, k, csl],
                            wabc_sb[:, i, k, :],
                            start=(k == 0), stop=(k == 1))
                evac(S[:, 0:2, c, 0:D], pab.rearrange("p (i d) -> p i d", i=2),
                     "SP")
                pwc = psum.tile([128, 512], f32, tag="big", name="pwc",
                                bufs=int(os.environ.get("KB_BIG", "4")))
                for k in (0, 1):
                    nc.tensor.matmul(pwc[:, 0:D], xts[:, k, csl],
                                     wabc_sb[:, 2, k, :],
                                     start=(k == 0), stop=(k == 1))
                evac(S[:, 2, c, 0:D], pwc[:, 0:D], "WC")

            # ---- attention strips for one shifted chunk (spans 2 blocks) ----
            strips = {}

            def emit_R(c):
                csl = slice(64 + c * 128, 64 + (c + 1) * 128)
                psr = psum.tile([128, 256], f32, tag="psr", name="psr",
                                bufs=int(os.environ.get("KB_PSR", "2")))
                for k in (0, 1):
                    nc.tensor.matmul(psr, xts[:, k, csl],
                                     qT[:, k, c * 128:c * 128 + 256],
                                     start=(k == 0), stop=False)
                nc.tensor.matmul(psr, ident_sb,
                                 bias_sb, start=False, stop=True,
                                 skip_group_check=True)
                ee = work.tile([128, 256], bf16, tag="ee", bufs=nstrip)
                nc.scalar.activation(ee, psr, AF.Exp)
                c2 = work.tile([128, 256], bf16, tag="c2", bufs=nstrip)
                c2eng = nc.gpsimd if os.environ.get("KB_C2", "pool") == "pool" else nc.vector
                c2eng.tensor_tensor(c2, ee, sucm_sb, op=OP.mult)
                c3 = work.tile([128, 256], bf16, tag="c3", bufs=nstrip)
                c3eng = nc.gpsimd if os.environ.get("KB_C3", "dve") == "pool" else nc.vector
                c3eng.scalar_tensor_tensor(
                    c3, in0=spk_bc[:, c * 128:c * 128 + 256],
                    scalar=spk_col[:, c:c + 1], in1=ee,
                    op0=OP.is_equal, op1=OP.mult)
                strips[c] = (ee, c2, c3)

            # ---- one 128-row output block ----
            psh_hist = {}

            def emit_block(b):
                eeA, c2A, c3A = strips[b]       # chunk b, block-b half = cols 128:256
                eeB, c2B, c3B = strips[b + 1]   # chunk b+1, block-b half = cols 0:128
                psh = psum.tile([128, 257], f32, tag="psh", name="psh",
                                bufs=int(os.environ.get("KB_PSH", "2")))
                # strip tile of chunk b exposes block b in its column half 1
                # (cols 128:256); chunk b+1 in half 0. The support rows always
                # come from the strip's own chunk.
                mms = [(eeA, 1, 0, 0), (eeB, 0, 1, 0), (c3A, 1, 0, 2),
                       (c3B, 0, 1, 2), (c2A, 1, 0, 1), (c2B, 0, 1, 1)]
                for i, (strip, half, coff, r) in enumerate(mms):
                    wid = 257 if r == 0 else D
                    nc.tensor.matmul(psh[:, 0:wid],
                                     strip[:, half * 128:(half + 1) * 128],
                                     S[:, r, b + coff, 0:wid],
                                     start=(i == 0), stop=(i == len(mms) - 1),
                                     skip_group_check=True)
                psh_hist[b] = psh
                rinv = rinv_all[:, b:b + 1]
                nc.vector.reciprocal(rinv, psh[:, 256:257])
                e2 = work.tile([128, D], bf16, tag="e2")
                nsolo = int(os.environ.get("KB_SOLO", "16"))
                if b >= NBLK - nsolo:
                    # tail blocks: finalize solo with the Act accumulator to
                    # shorten the end-of-kernel chain
                    nc.scalar.activation(e2, psh[:, 0:D], AF.Exp, scale=rinv,
                                         accum_out=s2_all[:, b:b + 1])
                    ln1 = work.tile([128, 1], f32, tag="ln2")
                    nc.scalar.activation(ln1, s2_all[:, b:b + 1], AF.Ln)
                    ob1 = work.tile([128, 1, D], f32, tag="ob2")
                    obeng = os.environ.get("KB_OBS", "dve")
                    if obeng == "act" or (obeng == "mixed" and b % 2 == 0):
                        bias1 = work.tile([128, 1], f32, tag="bias2")
                        nc.vector.tensor_scalar_mul(bias1, ln1, -1.0)
                        nc.scalar.activation(
                            ob1[:, 0, :], psh[:, 0:D], AF.Identity,
                            bias=bias1, scale=rinv)
                    else:
                        nc.vector.tensor_scalar(
                            ob1[:, 0, :], psh[:, 0:D], scalar1=rinv,
                            scalar2=ln1, op0=OP.mult, op1=OP.subtract)
                    nc.sync.dma_start(
                        out_d.rearrange("(c p) d -> p c d", p=128)[:, b:b + 1, :],
                        ob1)
                    return
                nc.scalar.activation(e2, psh[:, 0:D], AF.Exp, scale=rinv)
                nc.vector.reduce_sum(s2_all[:, b:b + 1], e2, axis=AX)

                if b % 2 == 1:
                    g = b // 2
                    gs = slice(g * 2, g * 2 + 2)
                    ln2 = work.tile([128, 2], f32, tag="ln2")
                    nc.scalar.activation(ln2, s2_all[:, gs], AF.Ln)
                    bias2 = work.tile([128, 2], f32, tag="bias2")
                    nc.vector.tensor_scalar_mul(bias2, ln2, -1.0)
                    ob2 = work.tile([128, 2, D], f32, tag="ob2")
                    obmode = os.environ.get("KB_OB", "mix")
                    for i in range(2):
                        bb = 2 * g + i
                        if obmode == "act" or (obmode == "mix" and i == 0):
                            nc.scalar.activation(
                                ob2[:, i, :], psh_hist[bb][:, 0:D], AF.Identity,
                                bias=bias2[:, i:i + 1],
                                scale=rinv_all[:, bb:bb + 1])
                        else:
                            nc.vector.tensor_scalar(
                                ob2[:, i, :], psh_hist[bb][:, 0:D],
                                scalar1=rinv_all[:, bb:bb + 1],
                                scalar2=bias2[:, i:i + 1],
                                op0=OP.mult, op1=OP.add)
                    nc.sync.dma_start(
                        out_d.rearrange("(c p) d -> p c d", p=128)[:, gs, :], ob2)

            # ---- interleaved driver: lookahead keeps PE fed while Act/DVE
            # produce the strips the next agg needs ----
            look = int(os.environ.get("KB_LOOK", "1"))
            emitted_s = {"n": 0, "q": 0}

            def advance(hi):
                hi = min(hi, NSH)
                while emitted_s["n"] < hi:
                    c = emitted_s["n"]
                    # R(c) reads qT data blocks c-1..c
                    while emitted_s["q"] <= BLK2QG[min(c, NBLK - 1)]:
                        emit_qT(emitted_s["q"])
                        emitted_s["q"] += 1
                    emit_S(c)
                    emit_R(c)
                    emitted_s["n"] += 1

            for b in range(NBLK):
                advance(b + 2 + look)
                emit_block(b)

    nc.compile()
    return nc


def _host_constants():
    import ml_dtypes
    jj = np.arange(128)[:, None]
    il = np.arange(128)[None, :]
    # Rc columns = [block c-1 (role B) | block c (role A)]
    bandA = jj >= il          # in-band for role A
    bandB = jj < il           # in-band for role B
    bias = np.concatenate([
        np.where(bandB, -SHIFT, NEG_BIG),
        np.where(bandA, -SHIFT, NEG_BIG)], axis=1).astype(np.float32)
    sucA = (il <= jj) & (jj < il + 64)
    sucB = jj < il - 64
    sucm = np.concatenate([sucB, sucA], axis=1).astype(ml_dtypes.bfloat16)
    ident = np.eye(128, dtype=np.float32)
    return bias, sucm, ident


def _prep_in_maps(np_inputs):
    import ml_dtypes
    bf16 = ml_dtypes.bfloat16
    x = np.asarray(np_inputs["x"], dtype=np.float32)
    spk = np.asarray(np_inputs["speaker_ids"]).astype(np.float32)
    W_att = np.asarray(np_inputs["W_att"], dtype=np.float32)
    W_pred = np.asarray(np_inputs["W_pred"], dtype=np.float32)
    W_suc = np.asarray(np_inputs["W_suc"], dtype=np.float32)
    W_same = np.asarray(np_inputs["W_same"], dtype=np.float32)
    W_diff = np.asarray(np_inputs["W_diff"], dtype=np.float32)

    bias, sucm, ident = _host_constants()
    wabc = np.stack([W_pred + W_diff, W_suc - W_pred, W_same - W_diff])

    xp = np.zeros((N_TOT + 2 * HALO, D), dtype=np.float32)
    xp[HALO:HALO + N_TOT] = x
    spkp = np.full((N_TOT + 2 * HALO,), -1.0, dtype=np.float32)
    spkp[HALO:HALO + N_TOT] = spk

    in_maps = []
    for k in range(NCORES):
        r0 = k * NC_ROWS
        spk_sh = spkp[r0:r0 + NH]
        spk_col = np.ascontiguousarray(
            spk_sh[64:64 + NSH * 128].reshape(NSH, 128).T)  # [128, 17]
        auxf = np.concatenate([bias, ident, spk_col], axis=1)  # [128, 401]
        spk_bc = np.broadcast_to(spk_sh.astype(bf16), (128, NH))
        auxb = np.concatenate([sucm, spk_bc], axis=1)          # [128, 2560]
        in_maps.append({
            "xt": np.ascontiguousarray(
                xp[r0:r0 + NH].T.reshape(2, 128, NH)),
            "wq": W_att, "wabc": wabc,
            "auxf": np.ascontiguousarray(auxf),
            "auxb": np.ascontiguousarray(auxb),
        })
    return in_maps


def kernel(x, speaker_ids, W_att, W_pred, W_suc, W_same, W_diff):
    from concourse import bass_utils

    if "nc" not in _cache:
        _cache["nc"] = _build_bass()
    nc = _cache["nc"]

    in_maps = _prep_in_maps({
        "x": x, "speaker_ids": speaker_ids, "W_att": W_att, "W_pred": W_pred,
        "W_suc": W_suc, "W_same": W_same, "W_diff": W_diff})

    res = bass_utils.run_bass_kernel_spmd(nc, in_maps, core_ids=list(range(NCORES)))
    _cache["last_result"] = res
    return np.concatenate([res.results[k]["out"] for k in range(NCORES)], axis=0)
